# revision 4
# baseline (speedup 1.0000x reference)
"""MoE text projection kernel for 8 TRN2 NeuronCores (Bass/Tile).

Problem: x[32,1024,768], gate_W[768,8], gate_b[8], expert_W[8,768,256],
expert_b[8,256] -> out[32,1024,256].  top-2 of 8 experts, softmax-over-all
gate, dense all-expert projection with masked weighted combine.

Strategy: data-parallel over tokens (32768 tokens -> 4096/core).  Host
pre-transposes x to xT[768, 4096] per core (contraction dim on partitions)
and rearranges expert_W; weights replicated.  On device per core:
  - gate logits in exact fp32 (top-2 selection is numerically sensitive),
  - softmax + top-2 mask via Max8 on VectorE,
  - all-8-expert projections in float32r (TF32-ish, 1 cyc/row) with PSUM
    accumulation over the 768-contraction,
  - weighted combine via per-partition-scalar fused multiply-add on VectorE,
  - expert-bias term via a tiny K=8 matmul (wm^T @ expert_b).
No collectives: outputs are disjoint token shards, host concatenates.
"""
import sys

sys.path.insert(0, "/opt/trn_rl_repo")

import numpy as np

# hardcoded problem shapes
BS, L, DIN, DOUT, E = 32, 1024, 768, 256, 8
NCORES = 8
NTOK = BS * L              # 32768
T = NTOK // NCORES         # 4096 tokens per core
KC = DIN // 128            # 6 contraction chunks
NG = 8                     # groups per core
TG = T // NG               # 512 tokens per group
NT = TG // 128             # 4 tiles per group

_STATE: dict = {}


def _build_program(reps: int = 1, use_act_round: bool = True,
                   expert_dtype: str = "f32r", dma_engine: str = "sync"):
    import concourse.mybir as mybir
    from concourse import bacc
    from concourse.tile import TileContext
    from concourse.masks import make_identity

    f32 = mybir.dt.float32
    f32r = (mybir.dt.float32r if expert_dtype == "f32r"
            else mybir.dt.bfloat16)

    nc = bacc.Bacc("TRN2", target_bir_lowering=False, debug=False,
                   num_devices=NCORES)
    xT_d = nc.dram_tensor("xt", [DIN, T], f32, kind="ExternalInput")
    gw_d = nc.dram_tensor("gw", [128, KC * E], f32, kind="ExternalInput")
    gb_d = nc.dram_tensor("gb", [128, NT * E], f32, kind="ExternalInput")
    ew_d = nc.dram_tensor("ew", [128, KC * E * DOUT], f32, kind="ExternalInput")
    eb_d = nc.dram_tensor("eb", [E, DOUT], f32, kind="ExternalInput")
    out_d = nc.dram_tensor("out", [T, DOUT], f32, kind="ExternalOutput")

    AL = mybir.AluOpType
    AF = mybir.ActivationFunctionType
    dma = nc.sync if dma_engine == "sync" else nc.gpsimd

    with TileContext(nc) as tc:
        with (
            tc.tile_pool(name="const", bufs=1) as cpool,
            tc.tile_pool(name="xg", bufs=2) as xg_pool,
            tc.tile_pool(name="xgr", bufs=2) as xgr_pool,
            tc.tile_pool(name="sm", bufs=4) as sm,
            tc.tile_pool(name="wm", bufs=2) as wm_pool,
            tc.tile_pool(name="wmt", bufs=2) as wmt_pool,
            tc.tile_pool(name="acc", bufs=3) as acc_pool,
            tc.tile_pool(name="pair", bufs=3, space="PSUM") as pair_ps,
            tc.tile_pool(name="gtw", bufs=1, space="PSUM") as gtw_ps,
            tc.tile_pool(name="gbk", bufs=1, space="PSUM") as gback_ps,
            tc.tile_pool(name="bps", bufs=1, space="PSUM") as b_ps,
            tc.tile_pool(name="wps", bufs=1, space="PSUM") as w_ps,
        ):
            ident = cpool.tile([128, 128], f32)
            make_identity(nc, ident)
            gw_sb = cpool.tile([128, KC * E], f32)
            gb_sb = cpool.tile([128, NT * E], f32)
            eb_sb = cpool.tile([E, DOUT], f32)
            eb_r = cpool.tile([E, DOUT], f32r)
            ew_r = cpool.tile([128, KC * E * DOUT], f32r)
            dma.dma_start(out=gw_sb, in_=gw_d[:, :])
            dma.dma_start(out=gb_sb, in_=gb_d[:, :])
            dma.dma_start(out=eb_sb, in_=eb_d[:, :])
            nc.vector.tensor_copy(eb_r, eb_sb)

            with tc.tile_pool(name="stage", bufs=1) as stage:
                ew_st = stage.tile([128, KC * E * DOUT], f32)
                dma.dma_start(out=ew_st, in_=ew_d[:, :])
                # round fp32 -> float32r for the TensorE fast path
                if use_act_round:
                    nc.scalar.copy(out=ew_r, in_=ew_st)
                else:
                    nc.vector.tensor_copy(ew_r, ew_st)

            def one_pass():
                for g in range(NG):
                    xg = xg_pool.tile([128, KC * TG], f32, tag="xg")
                    dma.dma_start(
                        out=xg.rearrange("p (k c) -> p k c", k=KC),
                        in_=xT_d.rearrange("(k p) t -> p k t", k=KC, p=128)
                        [:, :, g * TG:(g + 1) * TG],
                    )
                    xgr = xgr_pool.tile([128, KC * TG], f32r, tag="xgr")
                    if use_act_round:
                        nc.scalar.copy(out=xgr, in_=xg)
                    else:
                        nc.vector.tensor_copy(xgr, xg)

                    wm_g = wm_pool.tile([128, NT * E], f32, tag="wmg")
                    wps = w_ps.tile([8, NT * 128], f32, tag="wps")
                    # ---- gate, transposed: lgT[8, 512] in exact fp32 ----
                    gtp = gtw_ps.tile([8, TG], f32, tag="gtw")
                    for k in range(KC):
                        nc.tensor.matmul(
                            gtp,
                            gw_sb[:, k * E:(k + 1) * E],
                            xg[:, k * TG:(k + 1) * TG],
                            start=(k == 0), stop=(k == KC - 1),
                        )
                    lgT = sm.tile([8, TG], f32, tag="lgT")
                    nc.scalar.copy(out=lgT, in_=gtp)
                    # transpose back to [128 tok, 8] per tile
                    gbk = gback_ps.tile([128, NT * E], f32, tag="gbk")
                    for t in range(NT):
                        nc.tensor.transpose(
                            gbk[:, t * E:(t + 1) * E],
                            lgT[:, t * 128:(t + 1) * 128], ident[:8, :8])
                    lg_g = sm.tile([128, NT * E], f32, tag="lg")
                    nc.vector.tensor_add(lg_g, gbk, gb_sb)
                    ssum_g = sm.tile([128, NT], f32, tag="ssum")
                    rs_g = sm.tile([128, NT], f32, tag="rs")
                    for t in range(NT):
                        lg = lg_g[:, t * E:(t + 1) * E]
                        # ---- softmax + top-2 mask ----
                        m8 = sm.tile([128, 8], f32, tag="m8")
                        nc.vector.max(out=m8, in_=lg)
                        nm1 = sm.tile([128, 1], f32, tag="nm1")
                        nc.vector.tensor_scalar_mul(nm1, m8[:, 0:1], -1.0)
                        keep = sm.tile([128, E], f32, tag="keep")
                        nc.vector.tensor_scalar(
                            keep, lg, m8[:, 1:2], scalar2=None, op0=AL.is_ge)
                        texp = sm.tile([128, E], f32, tag="texp")
                        nc.scalar.activation(
                            texp, lg, AF.Exp, bias=nm1[:, 0:1], scale=1.0,
                            accum_out=ssum_g[:, t:t + 1])
                        # wm_pre = texp * keep (normalize after, batched)
                        nc.vector.tensor_mul(
                            wm_g[:, t * E:(t + 1) * E], texp, keep)
                    nc.vector.reciprocal(rs_g, ssum_g)
                    for t in range(NT):
                        # wm = wm_pre / s
                        nc.vector.tensor_scalar(
                            wm_g[:, t * E:(t + 1) * E],
                            wm_g[:, t * E:(t + 1) * E],
                            rs_g[:, t:t + 1], scalar2=None, op0=AL.mult)
                        # wm^T for the expert-bias matmul
                        nc.tensor.transpose(
                            wps[:, t * 128:(t + 1) * 128],
                            wm_g[:, t * E:(t + 1) * E], ident)

                    wmT_r = wmt_pool.tile([8, NT * 128], f32r, tag="wmt")
                    nc.vector.tensor_copy(wmT_r, wps)

                    bp = b_ps.tile([128, NT * DOUT], f32, tag="bp")
                    for t in range(NT):
                        nc.tensor.matmul(
                            bp[:, t * DOUT:(t + 1) * DOUT],
                            wmT_r[:, t * 128:(t + 1) * 128],
                            eb_r, start=True, stop=True)
                    acc_g = acc_pool.tile([128, NT * DOUT], f32, tag="acc")
                    if True:
                        for t in range(NT):
                            acc = acc_g[:, t * DOUT:(t + 1) * DOUT]
                            for pr in range(4):
                                pp = pair_ps.tile([128, 2 * DOUT], f32,
                                                  tag="pp", name=f"pp{pr}")
                                for k in range(KC):
                                    nc.tensor.matmul(
                                        pp,
                                        xgr[:, k * TG + t * 128: k * TG + (t + 1) * 128],
                                        ew_r[:, k * E * DOUT + 2 * pr * DOUT:
                                             k * E * DOUT + (2 * pr + 2) * DOUT],
                                        start=(k == 0), stop=(k == KC - 1),
                                    )
                                w0 = wm_g[:, t * E + 2 * pr: t * E + 2 * pr + 1]
                                w1 = wm_g[:, t * E + 2 * pr + 1: t * E + 2 * pr + 2]
                                if pr == 0:
                                    nc.vector.tensor_scalar(
                                        acc, pp[:, 0:DOUT], w0, scalar2=None,
                                        op0=AL.mult)
                                else:
                                    nc.vector.scalar_tensor_tensor(
                                        out=acc, in0=pp[:, 0:DOUT], scalar=w0,
                                        in1=acc, op0=AL.mult, op1=AL.add)
                                nc.vector.scalar_tensor_tensor(
                                    out=acc, in0=pp[:, DOUT:2 * DOUT], scalar=w1,
                                    in1=acc, op0=AL.mult, op1=AL.add)
                        nc.vector.tensor_add(acc_g, acc_g, bp)
                    dma.dma_start(
                        out=out_d.rearrange("(gg t p) n -> p (gg t) n", p=128, t=NT)
                        [:, g * NT:(g + 1) * NT, :],
                        in_=acc_g.rearrange("p (t n) -> p t n", t=NT),
                    )

            if reps == 1:
                one_pass()
            else:
                with tc.For_i(0, reps, 1):
                    one_pass()

    nc.compile()
    return nc


def _host_prep_weights(gate_W, gate_b, expert_W, expert_b):
    """Rearrange weights into the DMA-friendly layouts (replicated per core)."""
    gate_W = np.asarray(gate_W, dtype=np.float32)
    gate_b = np.asarray(gate_b, dtype=np.float32)
    expert_W = np.asarray(expert_W, dtype=np.float32)
    expert_b = np.asarray(expert_b, dtype=np.float32)
    # gw[p, k*8+j] = gate_W[k*128+p, j]
    gw = np.ascontiguousarray(
        gate_W.reshape(KC, 128, E).transpose(1, 0, 2).reshape(128, KC * E))
    gb = np.ascontiguousarray(np.tile(gate_b[None, :], (128, NT)))
    # ew[p, k*2048 + e*256 + n] = expert_W[e, k*128+p, n]
    ew = np.ascontiguousarray(
        expert_W.reshape(E, KC, 128, DOUT).transpose(2, 1, 0, 3)
        .reshape(128, KC * E * DOUT))
    eb = np.ascontiguousarray(expert_b)
    return gw, gb, ew, eb


def _get_runner(reps: int = 1, **build_kwargs):
    key = ("runner", reps, tuple(sorted(build_kwargs.items())))
    if key in _STATE:
        return _STATE[key]

    import jax
    from jax.sharding import Mesh, PartitionSpec
    from jax.experimental.shard_map import shard_map
    import concourse.mybir as mybir
    from concourse.bass2jax import (
        _bass_exec_p, install_neuronx_cc_hook, partition_id_tensor)

    nc = _build_program(reps=reps, **build_kwargs)
    install_neuronx_cc_hook()

    partition_name = (nc.partition_id_tensor.name
                      if nc.partition_id_tensor else None)
    in_names, out_names, out_avals = [], [], []
    for alloc in nc.m.functions[0].allocations:
        if not isinstance(alloc, mybir.MemoryLocationSet):
            continue
        name = alloc.memorylocations[0].name
        if alloc.kind == "ExternalInput":
            if name != partition_name:
                in_names.append(name)
        elif alloc.kind == "ExternalOutput":
            out_names.append(name)
            out_avals.append(jax.core.ShapedArray(
                tuple(alloc.tensor_shape), mybir.dt.np(alloc.dtype)))
    all_in_names = tuple(in_names) + tuple(out_names)
    if partition_name is not None:
        all_in_names = all_in_names + (partition_name,)
    n_params = len(in_names)

    def _body(*args):
        operands = list(args)
        if partition_name is not None:
            operands.append(partition_id_tensor())
        outs = _bass_exec_p.bind(
            *operands,
            out_avals=tuple(out_avals),
            in_names=all_in_names,
            out_names=tuple(out_names),
            lowering_input_output_aliases=(),
            sim_require_finite=True,
            sim_require_nnan=True,
            nc=nc,
        )
        return tuple(outs)

    devices = jax.devices()[:NCORES]
    mesh = Mesh(np.asarray(devices), ("core",))
    P = PartitionSpec("core")
    n_outs = len(out_names)
    fn = jax.jit(
        shard_map(_body, mesh=mesh,
                  in_specs=(P,) * (n_params + n_outs),
                  out_specs=(P,) * n_outs, check_rep=False),
        donate_argnums=tuple(range(n_params, n_params + n_outs)),
        keep_unused=True,
    )

    # On-device zero-buffer maker: the donated output args are produced on
    # device (memset), so steady-state calls transfer no host data at all.
    import jax.numpy as jnp
    from jax.sharding import NamedSharding

    sh = NamedSharding(mesh, P)

    def _mkzeros():
        return tuple(
            jnp.zeros((NCORES * a.shape[0], *a.shape[1:]), a.dtype)
            for a in out_avals)

    mkzeros = jax.jit(_mkzeros, out_shardings=(sh,) * n_outs)

    def fn2(*concat_in):
        return fn(*concat_in, *mkzeros())

    runner = {
        "nc": nc, "fn": fn, "fn2": fn2, "in_names": in_names,
        "out_names": out_names, "out_avals": out_avals, "mesh": mesh,
    }
    _STATE[key] = runner
    return runner


def _device_inputs(runner, cat):
    """device_put the concatenated inputs once per (runner, data) pair."""
    import jax
    from jax.sharding import NamedSharding, PartitionSpec

    key = ("dev_inputs", id(runner["fn2"]))
    if key in _STATE:
        return _STATE[key]
    sh = NamedSharding(runner["mesh"], PartitionSpec("core"))
    dev_in = [jax.device_put(cat[nm], sh) for nm in runner["in_names"]]
    _STATE[key] = dev_in
    return dev_in


def _make_concat_inputs(x, gate_W, gate_b, expert_W, expert_b):
    """Build the concatenated (8*dim0, ...) input arrays in in_names order."""
    x = np.asarray(x, dtype=np.float32)
    gw, gb, ew, eb = _host_prep_weights(gate_W, gate_b, expert_W, expert_b)
    toks = x.reshape(NTOK, DIN)
    # per-core transposed shards, stacked: xt_cat[c*DIN:(c+1)*DIN] = shard_c.T
    xt_cat = np.empty((NCORES * DIN, T), np.float32)
    for c in range(NCORES):
        xt_cat[c * DIN:(c + 1) * DIN] = toks[c * T:(c + 1) * T].T
    reps = {
        "xt": xt_cat,
        "gw": np.concatenate([gw] * NCORES, axis=0),
        "gb": np.concatenate([gb] * NCORES, axis=0),
        "ew": np.concatenate([ew] * NCORES, axis=0),
        "eb": np.concatenate([eb] * NCORES, axis=0),
    }
    return reps


def kernel(x, gate_W, gate_b, expert_W, expert_b):
    runner = _get_runner(reps=1)
    cat = _make_concat_inputs(x, gate_W, gate_b, expert_W, expert_b)
    concat_in = [cat[nm] for nm in runner["in_names"]]
    outs = runner["fn2"](*concat_in)
    out_cat = np.asarray(outs[runner["out_names"].index("out")])
    return out_cat.reshape(NCORES * T, DOUT).reshape(BS, L, DOUT)

